# revision 15
# baseline (speedup 1.0000x reference)
"""Trainium2 Bass kernel for nn_DenoiseGRU (bidirectional-GRU encoder +
attention GRU decoder + vocab projection).

Strategy (8 NeuronCores, SPMD, data-parallel over batch, BL=4 rows/core):
  - fp16 stationaries everywhere on the Tensor engine (FWL single-pass;
    fp32 stationaries lower to 2x LOW/HIGH passes at ~1.4us each -- avoided).
  - Precision-critical weights (dec Whh, attn_out_w) as fp16 hi+lo dual-rail
    (two accumulating matmul arms ~= f32-grade).
  - LayerNorm bypass: Whh @ h_new == rstd*(Whh @ y) - rstd*mu*(Whh @ 1),
    so the GRU matmuls consume y directly and the LN stats/rsqrt chain runs
    off the critical path. ln_g == 1, ln_b == 0 (fixed by the reference).
  - Softmax normalization deferred past the ctx matmuls (runs in parallel).
  - Vocab projection (512 x 32000) in fp16 computed per 32-step block,
    interleaved 2 vocab-chunks per decoder step to fill PE gaps; fp16 DRAM
    output upconverted on host.

Self-contained: hardcodes all shapes; no sibling imports.
"""

import sys
from contextlib import ExitStack

import numpy as np

sys.path.insert(0, "/opt/trn_rl_repo")

import concourse.bass as bass
import concourse.bacc as bacc
import concourse.tile as tile
from concourse import mybir
from concourse.bass_utils import run_bass_kernel_spmd

F32 = mybir.dt.float32
F32R = mybir.dt.float32r
F16 = mybir.dt.float16
I32 = mybir.dt.int32
AF = mybir.ActivationFunctionType
ALU = mybir.AluOpType

D_E, D_ENC, D_DEC, H, HD = 128, 256, 512, 8, 64
B, S, T, V, C = 32, 128, 128, 32000, 2
NCORES = 8
BL = B // NCORES      # 4 local batch rows per core
VC = 500              # vocab chunk (PSUM bank = 500 f32)
NVC = V // VC         # 64
TBLK = 32             # decoder steps per projection block
NBLK = T // TBLK      # 4
HERON_ITERS = 3


def _f16(x):
    return np.asarray(x, np.float32).astype(np.float16)


def _hi_lo(x):
    """Split f32 array into fp16 hi + fp16 lo(residual)."""
    x = np.asarray(x, np.float32)
    hi = x.astype(np.float16)
    lo = (x - hi.astype(np.float32)).astype(np.float16)
    return hi, lo


def build_program(dec_steps=T, enc_steps=S, debug_outs=False, finalize=True):
    nc = bacc.Bacc()

    def din(name, shape, dtype=F32):
        return nc.dram_tensor(name, shape, dtype, kind="ExternalInput")

    # ---------------- DRAM inputs (per-core) ----------------
    embT = din("embT", [D_E, S, BL], F32R)     # encoder emb^T (d, s, b)
    embT_rev = din("embT_rev", [D_E, S, BL], F32R)  # s-reversed copy
    dembT = din("dembT", [D_E, T, BL], F32R)   # decoder input emb^T (d, t, b)
    h0T = din("h0T", [128, 4, BL])             # decoder h_init^T (p, c, b) f32
    h0T_16 = din("h0T_16", [128, 4, BL], F16)
    rmu0 = din("rmu0", [1, 2, BL])             # (mu=0, rstd=1) init

    enc_wihT = {d: din(f"enc_wihT_{d}", [D_E, 6 * 128], F32R) for d in "fb"}
    enc_whh_hi = {d: din(f"enc_whh_hi_{d}", [128, 2, 6 * 128], F16) for d in "fb"}
    enc_whh_lo = {d: din(f"enc_whh_lo_{d}", [128, 2, 6 * 128], F16) for d in "fb"}
    enc_gbias = {d: din(f"enc_gbias_{d}", [1, 6 * 128], F32R) for d in "fb"}
    enc_bhn = {d: din(f"enc_bhn_{d}", [1, 2 * 128], F16) for d in "fb"}

    dec_wihT = din("dec_wihT", [D_E, 12 * 128], F32R)
    dec_whh_hi = din("dec_whh_hi", [128, 4, 12 * 128], F16)
    dec_whh_lo = din("dec_whh_lo", [128, 4, 12 * 128], F16)
    dec_gbias = din("dec_gbias", [1, 12 * 128], F32R)
    dec_uT = din("dec_uT", [128, 12])          # Whh row sums (p, m) f32
    dec_bhnB = din("dec_bhnB", [128, 4, BL])   # bhh n-part broadcast f32
    boutB = din("boutB", [128, 4, BL])         # attn_out_b broadcast f32

    wqT = din("wqT", [128, 4, 512], F16)
    bq_row = din("bq_row", [1, 512], F16)
    wout_hi = din("wout_hi", [128, 4, 512], F16)
    wout_lo = din("wout_lo", [128, 4, 512], F16)
    wkT = din("wkT", [128, 4, 512], F16)
    wvT = din("wvT", [128, 4, 512], F16)
    bk_row = din("bk_row", [1, 512], F16)
    bv_row = din("bv_row", [1, 512], F16)
    masks = din("masks", [128, 2], F16)        # lo/hi head masks
    onesN16 = din("onesN16", [1, 512], F16)
    ones128_16 = din("ones128_16", [128, 1], F16)
    ones512_r = din("ones512_r", [1, 512], F32R)
    onesM_r = din("onesM_r", [1, 128], F32R)   # f32r ones row (bcast stat.)
    ones128_r = din("ones128_r", [128, 1], F32R)

    projT = din("projT", [128, 4, V], F16)     # proj_w^T packed (p, c, v)
    projb = din("projb", [1, V], F16)

    out_v = nc.dram_tensor("out_v", [BL, T, V], F16, kind="ExternalOutput")
    dbg = {}
    if debug_outs:
        dbg["mem"] = nc.dram_tensor("dbg_mem", [128, 4, BL, S], F16, kind="ExternalOutput")
        dbg["kT"] = nc.dram_tensor("dbg_kT", [128, 4, BL, S], F16, kind="ExternalOutput")
        dbg["v"] = nc.dram_tensor("dbg_v", [128, BL, 512], F16, kind="ExternalOutput")
        dbg["hnew"] = nc.dram_tensor("dbg_hnew", [128, 4, T, BL], F16, kind="ExternalOutput")

    with tile.TileContext(nc) as tc, ExitStack() as ctx:
        P = ctx.enter_context(tc.tile_pool(name="persist", bufs=1))
        W = ctx.enter_context(tc.tile_pool(name="work", bufs=3))
        PS = ctx.enter_context(tc.tile_pool(name="psum", bufs=1, space="PSUM"))
        PSB = ctx.enter_context(tc.tile_pool(name="psumb", bufs=2, space="PSUM"))
        PBG = ctx.enter_context(tc.tile_pool(name="psumg", bufs=2, space="PSUM"))
        PSJ = ctx.enter_context(tc.tile_pool(name="psumj", bufs=2, space="PSUM"))
        PWP = ctx.enter_context(tc.tile_pool(name="pw", bufs=4))

        dma = nc.sync.dma_start

        # ---------------- SBUF persistent tiles + loads ----------------
        def load(name, dram, shape, dtype):
            t = P.tile(shape, dtype, tag=name)
            dma(out=t, in_=dram)
            return t

        s_embT = load("s_embT", embT[:, :, :], [D_E, S, BL], F32R)
        s_embT_rev = load("s_embT_rev", embT_rev[:, :, :], [D_E, S, BL], F32R)
        s_dembT = load("s_dembT", dembT[:, :, :], [D_E, T, BL], F32R)
        s_h0 = load("s_h0", h0T[:, :, :], [128, 4, BL], F32)
        s_h016 = load("s_h016", h0T_16[:, :, :], [128, 4, BL], F16)
        s_rmu0 = load("s_rmu0", rmu0[:, :, :], [1, 2, BL], F32)
        s_ewihT = {d: load(f"s_ewihT_{d}", enc_wihT[d][:, :], [D_E, 768], F32R) for d in "fb"}
        s_ewhh_hi = {d: load(f"s_ewhh_hi_{d}", enc_whh_hi[d][:, :, :], [128, 2, 768], F16) for d in "fb"}
        s_ewhh_lo = {d: load(f"s_ewhh_lo_{d}", enc_whh_lo[d][:, :, :], [128, 2, 768], F16) for d in "fb"}
        s_egb = {d: load(f"s_egb_{d}", enc_gbias[d][:, :], [1, 768], F32R) for d in "fb"}
        s_ebhn = {d: load(f"s_ebhn_{d}", enc_bhn[d][:, :], [1, 256], F16) for d in "fb"}
        s_dwihT = load("s_dwihT", dec_wihT[:, :], [D_E, 1536], F32R)
        s_dwhh_hi = load("s_dwhh_hi", dec_whh_hi[:, :, :], [128, 4, 1536], F16)
        s_dwhh_lo = load("s_dwhh_lo", dec_whh_lo[:, :, :], [128, 4, 1536], F16)
        s_dgb = load("s_dgb", dec_gbias[:, :], [1, 1536], F32R)
        s_u = load("s_u", dec_uT[:, :], [128, 12], F32)
        s_bhnB = load("s_bhnB", dec_bhnB[:, :, :], [128, 4, BL], F32)
        s_boutB = load("s_boutB", boutB[:, :, :], [128, 4, BL], F32)
        s_wq = load("s_wq", wqT[:, :, :], [128, 4, 512], F16)
        s_bq = load("s_bq", bq_row[:, :], [1, 512], F16)
        s_wout_hi = load("s_wout_hi", wout_hi[:, :, :], [128, 4, 512], F16)
        s_wout_lo = load("s_wout_lo", wout_lo[:, :, :], [128, 4, 512], F16)
        s_wk = load("s_wk", wkT[:, :, :], [128, 4, 512], F16)
        s_wv = load("s_wv", wvT[:, :, :], [128, 4, 512], F16)
        s_bk = load("s_bk", bk_row[:, :], [1, 512], F16)
        s_bv = load("s_bv", bv_row[:, :], [1, 512], F16)
        s_masks = load("s_masks", masks[:, :], [128, 2], F16)
        s_onesN = load("s_onesN", onesN16[:, :], [1, 512], F16)
        s_ones128_16 = load("s_ones128_16", ones128_16[:, :], [128, 1], F16)
        s_ones512_r = load("s_ones512_r", ones512_r[:, :], [1, 512], F32R)
        s_onesM_r = load("s_onesM_r", onesM_r[:, :], [1, 128], F32R)
        s_ones128_r = load("s_ones128_r", ones128_r[:, :], [128, 1], F32R)

        # ---- persistent state/stores ----
        s_giE = P.tile([128, 12, S, BL], F16, tag="s_giE")
        POS = {"f": [0, 1, 4, 5, 8, 9], "b": [2, 3, 6, 7, 10, 11]}
        s_gid = P.tile([128, 12, T, BL], F32, tag="s_gid")
        s_mem16 = P.tile([128, 4, BL, S], F16, tag="s_mem16")   # (p, c, b, s)
        s_kT = P.tile([128, 4, BL, S], F16, tag="s_kT")
        s_v16 = P.tile([128, BL, 512], F16, tag="s_v16")        # (p=s, b, vdim)
        s_hz = P.tile([128, 2, BL], F16, tag="s_hz")            # zero carry
        s_h = P.tile([128, 4, BL], F32, tag="s_h")              # h_new[t-1] f32
        s_y16 = P.tile([128, 4, BL], F16, tag="s_y16")          # y[t-1] fp16
        s_y16l = P.tile([128, 4, BL], F16, tag="s_y16l")        # y residual
        s_ys2 = P.tile([128, 2, 4, BL], F32, tag="s_ys2")       # y, y^2
        s_ysr = P.tile([128, 2, 4, BL], F32, tag="s_ysr")       # y, y^2 (f32r)
        s_rmu = P.tile([1, 2, BL], F32, tag="s_rmu")            # (mu, rstd)
        s_rmu16h = P.tile([1, 2, BL], F16, tag="s_rmu16h")
        s_rmu16l = P.tile([1, 2, BL], F16, tag="s_rmu16l")
        s_hb = [P.tile([128, 4, TBLK, BL], F16, name=f"s_hb{i}", tag=f"s_hb{i}")
                for i in range(NBLK)]
        s_fisK = P.tile([1, BL], I32, tag="s_fisK")

        MM = nc.tensor.matmul
        TT = nc.vector.tensor_tensor
        CP = nc.vector.tensor_copy
        STT = nc.vector.scalar_tensor_tensor

        def TS(out, in0, s1, s2, op0, op1=None):
            if op1 is None:
                nc.vector.tensor_scalar(out, in0, s1, None, op0)
            else:
                nc.vector.tensor_scalar(out, in0, s1, s2, op0, op1)

        def r32(ap):
            return ap.bitcast(F32R)

        def mkap(ap_, free_dims):
            """Replace the free dims of a (tile-slice) AP with explicit
            [step, count] entries (steps in elements; 0 = broadcast)."""
            return bass.AP(tensor=ap_.tensor, offset=ap_.offset,
                           ap=[list(ap_.ap[0])] + [list(x) for x in free_dims])

        nc.vector._memset_packed(s_fisK, 0x5F3759DF)
        nc.vector._memset_packed(
            s_hz.rearrange("p c b -> p (c b)").bitcast(I32), 0)

        # ================ phase 1: gi builds (f32r matmuls -> fp16 stores) ==
        def build_gi(wihT_s, gb_s, src, dest, nm, nt, posmap=None):
            # dest[:, m, t, b] = (Wih @ x^T)[128m:128m+128, (t,b)] + bias
            for m in range(nm):
                pb = PBG.tile([128, nt * BL], F32, name="pb", tag="pbig")
                MM(pb, gb_s[:, 128 * m:128 * (m + 1)],
                   s_ones512_r[:, :nt * BL], start=True, stop=False)
                MM(pb, wihT_s[:, 128 * m:128 * (m + 1)],
                   src.rearrange("d t b -> d (t b)"), start=False, stop=True)
                mo = posmap[m] if posmap else m
                CP(dest[:, mo, :, :].rearrange("p t b -> p (t b)"), pb)

        build_gi(s_ewihT["f"], s_egb["f"], s_embT, s_giE, 6, S, POS["f"])
        build_gi(s_ewihT["b"], s_egb["b"], s_embT_rev, s_giE, 6, S, POS["b"])
        build_gi(s_dwihT, s_dgb, s_dembT, s_gid, 12, T)

        # ================ phase 2: encoder scans ================
        def enc_mms(d, t, pghe):
            # matmuls for one direction into the merged psum layout:
            # positions [f_r b_r f_z b_z f_n b_n] (2 chunks each)
            whh_hi, whh_lo = s_ewhh_hi[d], s_ewhh_lo[d]
            off = 0 if d == "f" else 2
            if t == 0:
                hmov = s_hz[:, :, :]
            else:
                prev = t - 1 if d == "f" else S - t
                hmov = s_mem16[:, off:off + 2, :, prev]
            pos = POS[d]
            for m in range(6):
                mo = pos[m]
                if m >= 4:
                    MM(pghe[:, mo, :], s_ebhn[d][:, 128 * (m - 4):128 * (m - 3)],
                       s_onesN[:, :BL], start=True, stop=False)
                for arm, wt in ((0, whh_hi), (1, whh_lo)):
                    for k in range(2):
                        MM(pghe[:, mo, :], wt[:, k, 128 * m:128 * (m + 1)],
                           hmov[:, k, :],
                           start=(m < 4 and arm == 0 and k == 0),
                           stop=(arm == 1 and k == 1))
            return hmov

        def enc_gates(t, pghe, hmov_f, hmov_b):
            pre8 = W.tile([128, 8, BL], F32, tag="e_pre8")
            TT(pre8, pghe[:, 0:8, :], s_giE[:, 0:8, t, :], ALU.add)
            rzt = W.tile([128, 8, BL], F32, tag="e_rzt")
            nc.scalar.activation(rzt, pre8, AF.Tanh, scale=0.5)
            tmp4 = W.tile([128, 4, BL], F32, tag="e_tmp4")
            STT(tmp4, rzt[:, 0:4, :], 1.0, pghe[:, 8:12, :], ALU.add, ALU.mult)
            tn4 = W.tile([128, 4, BL], F32, tag="e_tn4")
            STT(tn4, tmp4, 0.5, s_giE[:, 8:12, t, :], ALU.mult, ALU.add)
            nn4 = W.tile([128, 4, BL], F32, tag="e_nn4")
            nc.scalar.activation(nn4, tn4, AF.Tanh)
            for d, hmov in (("f", hmov_f), ("b", hmov_b)):
                off = 0 if d == "f" else 2
                srcpos = t if d == "f" else S - 1 - t
                no = slice(0, 2) if d == "f" else slice(2, 4)
                zo = slice(4, 6) if d == "f" else slice(6, 8)
                dd = W.tile([128, 2, BL], F32, name=f"dd_{d}", tag=f"e_d_{d}")
                TT(dd, hmov, nn4[:, no, :], ALU.subtract)
                STT(dd, rzt[:, zo, :], 1.0, dd, ALU.add, ALU.mult)
                STT(s_mem16[:, off:off + 2, :, srcpos], dd, 0.5,
                    nn4[:, no, :], ALU.mult, ALU.add)

        if enc_steps < S:
            nc.vector._memset_packed(
                s_mem16.rearrange("p c b s -> p (c b s)").bitcast(I32), 0)
        for t in range(enc_steps):
            pghe = PS.tile([128, 12, BL], F32, name="pghe", tag="pghe")
            hf = enc_mms("f", t, pghe)
            hb = enc_mms("b", t, pghe)
            enc_gates(t, pghe, hf, hb)

        # ================ phase 3: k / v builds ================
        for m in range(4):
            pb = PBG.tile([128, BL * S], F32, name="pb", tag="pbig")
            MM(pb, s_bk[:, 128 * m:128 * (m + 1)], s_onesN, start=True, stop=False)
            for k in range(4):
                MM(pb, s_wk[:, k, 128 * m:128 * (m + 1)],
                   s_mem16[:, k, :, :].rearrange("p b s -> p (b s)"),
                   start=False, stop=(k == 3))
            CP(s_kT[:, m, :, :].rearrange("p b s -> p (b s)"), pb)
            if debug_outs:
                dma(out=dbg["kT"][:, m, :, :], in_=s_kT[:, m, :, :])
        for g in range(BL):
            pb = PBG.tile([128, 512], F32, name="pb", tag="pbig")
            MM(pb, s_onesN[:, 0:128], s_bv, start=True, stop=False)
            for k in range(4):
                MM(pb, s_mem16[:, k, g, :], s_wv[:, k, :], start=False, stop=(k == 3))
            CP(s_v16[:, g, :], pb)
            if debug_outs:
                dma(out=dbg["v"][:, g, :], in_=s_v16[:, g, :])
        if debug_outs:
            dma(out=dbg["mem"].rearrange("p c b s -> p (c b s)"),
                in_=s_mem16.rearrange("p c b s -> p (c b s)"))

        # ================ phase 4+5: decoder + interleaved projection =======
        RECIP = nc.vector.reciprocal

        # init: y[-1] := h0 (with mu=0, rstd=1 so that h_new[-1] = h0)
        CP(s_h, s_h0)
        CP(s_y16, s_h016)
        TT(s_y16l, s_h0, s_h016, ALU.subtract)
        CP(s_rmu, s_rmu0)

        def gru_mms(t, m0, m1):
            # hi+lo arm matmuls for m-chunks [m0, m1) on y[t-1]
            pgh = dec_state["pgh"]
            for m in range(m0, m1):
                for arm, wt, ym in ((0, s_dwhh_hi, s_y16), (1, s_dwhh_lo, s_y16),
                                    (2, s_dwhh_hi, s_y16l)):
                    for k in range(4):
                        MM(pgh[:, m, :], wt[:, k, 128 * m:128 * (m + 1)],
                           ym[:, k, :],
                           start=(arm == 0 and k == 0),
                           stop=(arm == 2 and k == 3))

        def stats_mms(pbcx):
            # partition-sums of (y, y^2) for the previous step
            pst2 = pbcx[0:1, 40:48].rearrange("p (a b) -> p a b", a=2)
            for c in range(4):
                MM(pst2, s_ones128_r, r32(s_ysr[:, :, c, :]),
                   start=(c == 0), stop=(c == 3))
            return pst2

        def stats_chain(pst2):
            # mu, rstd <- pst2  (DVE, off critical path)
            stA = W.tile([1, 2, BL], F32, tag="d_stA")
            TS(stA, pst2, 1.0 / D_DEC, None, ALU.mult)
            CP(s_rmu[:, 0, :], stA[:, 0, :])
            varE = W.tile([1, BL], F32, tag="d_varE")
            STT(varE, stA[:, 0, :], 1.0, stA[:, 0, :], ALU.mult, ALU.mult)
            TT(varE, stA[:, 1, :], varE, ALU.subtract)
            TS(varE, varE, 1e-5, None, ALU.add)
            zz = W.tile([1, BL], F32, tag="d_zz")
            TS(zz.bitcast(I32), varE.bitcast(I32), 1, None,
               ALU.logical_shift_right)
            TT(zz.bitcast(I32), s_fisK.bitcast(I32), zz.bitcast(I32),
               ALU.subtract)
            dtmp = W.tile([1, BL], F32, tag="d_dtmp")
            for it in range(HERON_ITERS):
                TT(dtmp, zz, zz, ALU.mult)
                TT(dtmp, varE, dtmp, ALU.mult)
                TS(dtmp, dtmp, -0.5, 1.5, ALU.mult, ALU.add)
                if it == HERON_ITERS - 1:
                    TT(s_rmu[:, 1, :], zz, dtmp, ALU.mult)
                else:
                    TT(zz, zz, dtmp, ALU.mult)

        def bcast_rmu(pbcx):
            CP(s_rmu16h, s_rmu)
            TT(s_rmu16l, s_rmu, s_rmu16h, ALU.subtract)
            pbc2 = pbcx[:, 0:8].rearrange("p (a b) -> p a b", a=2)
            MM(pbc2.rearrange("p a b -> p (a b)"), s_onesN[:, 0:128],
               s_rmu16h.rearrange("p a b -> p (a b)"), start=True, stop=False)
            MM(pbc2.rearrange("p a b -> p (a b)"), s_onesN[:, 0:128],
               s_rmu16l.rearrange("p a b -> p (a b)"), start=False, stop=True)
            return pbc2

        def materialize_h(t, rmuB):
            # h_new[t] = (y[t] - mu)*rstd  -> s_h (f32) and block store (fp16)
            ytmp = W.tile([128, 4, BL], F32, tag="d_ytmp")
            TT(ytmp, s_ys2[:, 0, :, :], mkap(rmuB[:, 0, :], [[0, 4], [1, BL]]),
               ALU.subtract)
            TT(s_h, ytmp, mkap(rmuB[:, 1, :], [[0, 4], [1, BL]]), ALU.mult)
            CP(s_hb[t // TBLK][:, :, t % TBLK, :], s_h)

        def proj_vc(blk, vc):
            s_pw = PWP.tile([128, 4, VC], F16, tag="s_pw")
            dma(out=s_pw, in_=projT[:, :, VC * vc:VC * (vc + 1)])
            s_pb = PWP.tile([1, VC], F16, tag="s_pb")
            dma(out=s_pb, in_=projb[:, VC * vc:VC * (vc + 1)])
            pj = PSJ.tile([128, VC], F32, tag="pproj")
            MM(pj, s_onesN[:, 0:128], s_pb, start=True, stop=False)
            for c in range(4):
                MM(pj, s_hb[blk][:, c, :, :].rearrange("p t b -> p (t b)"),
                   s_pw[:, c, :], start=False, stop=(c == 3))
            so = PWP.tile([128, VC], F16, tag="s_out")
            CP(so, pj)
            r0 = blk * TBLK
            dma(out=out_v[:, r0:r0 + TBLK, VC * vc:VC * (vc + 1)]
                .rearrange("b t v -> t b v"), in_=so)

        dec_state = {}
        for t in range(dec_steps):
            # ---- GRU matmuls on y[t-1] (start immediately; only need y16) --
            pgh = PS.tile([128, 12, BL], F32, name="pgh", tag="pghe")
            dec_state["pgh"] = pgh
            patt = PS.tile([128, 128], F32, name="patt", tag="patt")
            pbcx = PSB.tile([128, 64], F32, name="pbcx", tag="pbcx")
            gru_mms(t, 0, 4)
            pst2 = stats_mms(pbcx) if t > 0 else None
            gru_mms(t, 4, 10)
            if t > 0:
                stats_chain(pst2)           # DVE, overlaps the m=4..9 MMs
            pbc2 = bcast_rmu(pbcx)          # PE; waits on s_rmu (newton)
            gru_mms(t, 10, 12)
            rmuB = W.tile([128, 2, BL], F32, name="rmuB", tag="d_rmuB")
            CP(rmuB, pbc2)
            muB = rmuB[:, 0, :]
            rstdB = rmuB[:, 1, :]
            # ---- corrections (DVE, overlap the GRU tail) ----
            rm = W.tile([128, BL], F32, tag="d_rm")
            TT(rm, muB, rstdB, ALU.mult)
            uw = W.tile([128, 12, BL], F32, tag="d_uw")
            TT(uw, mkap(s_u, [[1, 12], [0, BL]]),
               mkap(rm, [[0, 12], [1, BL]]), ALU.mult)
            giA = W.tile([128, 8, BL], F32, tag="d_giA")
            TT(giA, s_gid[:, 0:8, t, :], uw[:, 0:8, :], ALU.subtract)
            Cn = W.tile([128, 4, BL], F32, tag="d_Cn")
            TT(Cn, s_bhnB, uw[:, 8:12, :], ALU.subtract)
            if t > 0:
                materialize_h(t - 1, rmuB)
            # ---- gates ----
            pghS = W.tile([128, 12, BL], F32, tag="d_pghS")
            TT(pghS, pgh, mkap(rstdB, [[0, 12], [1, BL]]), ALU.mult)
            pre8 = W.tile([128, 8, BL], F32, tag="d_pre8")
            TT(pre8, pghS[:, 0:8, :], giA, ALU.add)
            npre = W.tile([128, 4, BL], F32, tag="d_npre")
            TT(npre, pghS[:, 8:12, :], Cn, ALU.add)
            rzt = W.tile([128, 8, BL], F32, tag="d_rzt")
            nc.scalar.activation(rzt, pre8, AF.Tanh, scale=0.5)
            tn = W.tile([128, 4, BL], F32, tag="d_tn")
            STT(tn, rzt[:, 0:4, :], 1.0, npre, ALU.add, ALU.mult)
            STT(tn, tn, 0.5, s_gid[:, 8:12, t, :], ALU.mult, ALU.add)
            nn_ = W.tile([128, 4, BL], F32, tag="d_n")
            nc.scalar.activation(nn_, tn, AF.Tanh)
            dd = W.tile([128, 4, BL], F32, tag="d_dd")
            TT(dd, s_h, nn_, ALU.subtract)
            STT(dd, rzt[:, 4:8, :], 1.0, dd, ALU.add, ALU.mult)
            hdecT = W.tile([128, 4, BL], F32, tag="d_hdec")
            STT(hdecT, dd, 0.5, nn_, ALU.mult, ALU.add)
            hdec16 = W.tile([128, 4, BL], F16, tag="d_hdec16")
            CP(hdec16, hdecT)
            # ---- q = Wq @ h_dec + bq ----
            pq = patt[:, 0:16].rearrange("p (c b) -> p c b", c=4)
            for m in range(4):
                MM(pq[:, m, :], s_bq[:, 128 * m:128 * (m + 1)],
                   s_onesN[:, :BL], start=True, stop=False)
                for k in range(4):
                    MM(pq[:, m, :], s_wq[:, k, 128 * m:128 * (m + 1)],
                       hdec16[:, k, :], start=False, stop=(k == 3))
            # ---- masked q ----
            mq = W.tile([128, 4, BL, 2], F16, tag="d_mq")
            TT(mq, pq.to_broadcast([128, 4, BL, 2]),
               mkap(s_masks, [[0, 4], [0, BL], [1, 2]]), ALU.mult)
            # ---- scores^T ----
            psc = patt[:, 16:48].rearrange("p (b c j) -> p b c j", b=BL, c=4)
            for b in range(BL):
                for c in range(4):
                    MM(psc[:, b, c, :], s_kT[:, c, b, :], mq[:, c, b, :],
                       start=True, stop=True)
            # ---- exp (unnormalized); denominator+recip run beside ctx ----
            attnE = W.tile([128, BL, 4, 2], F16, tag="d_attnE")
            nc.scalar.activation(attnE.rearrange("p b c j -> p (b c j)"),
                                 psc.rearrange("p b c j -> p (b c j)"),
                                 AF.Exp, scale=0.125)
            pst = patt[0:1, 48:80]
            MM(pst, s_ones128_16, attnE.rearrange("p b c j -> p (b c j)"),
               start=True, stop=True)
            rec = W.tile([1, 32], F32, tag="d_rec")
            with nc.allow_low_precision(reason="f32r recip for bcast mm"):
                RECIP(r32(rec), pst)
            pbc = pbcx[:, 8:40]
            MM(pbc, s_onesM_r, r32(rec), start=True, stop=True)
            recB = W.tile([128, BL, 4, 2], F32, tag="d_recB")
            CP(recB.rearrange("p b c j -> p (b c j)"), pbc)
            pctx = patt[:, 80:112].rearrange("p (b c j) -> p b c j", b=BL, c=4)
            for b in range(BL):
                for c in range(4):
                    MM(pctx[:, b, c, :], s_v16[:, b, 128 * c:128 * (c + 1)],
                       attnE[:, b, c, :], start=True, stop=True)
            # ---- normalize + head-merge: ctxb[p, c, b] (fp16) ----
            ctxf = W.tile([128, 4, BL], F32, tag="d_ctxf")
            TT(ctxf[0:64, :, :].rearrange("p c b -> p b c"),
               pctx[0:64, :, :, 0], recB[0:64, :, :, 0], ALU.mult)
            TT(ctxf[64:128, :, :].rearrange("p c b -> p b c"),
               pctx[64:128, :, :, 1], recB[64:128, :, :, 1], ALU.mult)
            ctxh = W.tile([128, 4, BL], F16, tag="d_ctxh")
            CP(ctxh, ctxf)
            ctxl = W.tile([128, 4, BL], F16, tag="d_ctxl")
            TT(ctxl, ctxf, ctxh, ALU.subtract)
            # ---- c = Wout @ ctx (3 arms: hi*hi + hi*lo + lo*hi) ----
            pc = patt[:, 112:128].rearrange("p (c b) -> p c b", c=4)
            for m in range(4):
                for arm, wt, cx in ((0, s_wout_hi, ctxh), (1, s_wout_hi, ctxl),
                                    (2, s_wout_lo, ctxh)):
                    for k in range(4):
                        MM(pc[:, m, :], wt[:, k, 128 * m:128 * (m + 1)],
                           cx[:, k, :],
                           start=(arm == 0 and k == 0),
                           stop=(arm == 2 and k == 3))
            # ---- y = hdec + c + bout ; y^2 ----
            TT(s_ys2[:, 0, :, :], hdecT, pc, ALU.add)
            TT(s_ys2[:, 0, :, :], s_ys2[:, 0, :, :], s_boutB, ALU.add)
            CP(s_y16, s_ys2[:, 0, :, :])
            TT(s_y16l, s_ys2[:, 0, :, :], s_y16, ALU.subtract)
            TT(r32(s_ysr[:, 1, :, :]), s_ys2[:, 0, :, :], s_ys2[:, 0, :, :], ALU.mult)
            CP(r32(s_ysr[:, 0, :, :]), s_ys2[:, 0, :, :])
            # ---- interleaved projection: 2 vocab chunks of previous block --
            if t >= TBLK:
                pblk = t // TBLK - 1
                voff = 2 * (t % TBLK)
                proj_vc(pblk, voff)
                proj_vc(pblk, voff + 1)

        # ---- epilogue: stats for final y, materialize last h, last block ---
        pbcx = PSB.tile([128, 64], F32, name="pbcx", tag="pbcx")
        pst2 = stats_mms(pbcx)
        stats_chain(pst2)
        pbc2 = bcast_rmu(pbcx)
        rmuB = W.tile([128, 2, BL], F32, name="rmuB_f", tag="d_rmuB")
        CP(rmuB, pbc2)
        materialize_h(dec_steps - 1, rmuB)
        lastblk = (dec_steps - 1) // TBLK
        for vc in range(NVC):
            proj_vc(lastblk, vc)
        if debug_outs:
            for i in range(NBLK):
                dma(out=dbg["hnew"][:, :, TBLK * i:TBLK * (i + 1), :],
                    in_=s_hb[i])

    if finalize:
        nc.compile()
    return nc


# ---------------------------------------------------------------------------
# Host driver
# ---------------------------------------------------------------------------

def _prep_core_inputs(inp, core):
    """Build the per-core input map (numpy) for batch slice [4*core, 4*core+4)."""
    f32 = np.float32
    sl = slice(BL * core, BL * (core + 1))
    nx = np.asarray(inp["nx"])[sl]
    x = np.asarray(inp["x"])[sl]
    label = np.asarray(inp["label"])[sl]
    tok = np.asarray(inp["tok_emb"], f32)

    m = {}
    emb = tok[nx]  # [BL, S, D_E]
    m["embT"] = np.ascontiguousarray(emb.transpose(2, 1, 0))  # (d, s, b)
    m["embT_rev"] = np.ascontiguousarray(emb[:, ::-1].transpose(2, 1, 0))
    dec_in = np.concatenate(
        [np.broadcast_to(np.asarray(inp["start_emb"], f32)[None], (BL, 1, D_E)),
         tok[x[:, :-1]]], axis=1)
    m["dembT"] = np.ascontiguousarray(dec_in.transpose(2, 1, 0))
    h0 = np.asarray(inp["style_emb"], f32)[label]  # [BL, 512]
    h0T = np.ascontiguousarray(h0.T.reshape(4, 128, BL).transpose(1, 0, 2))
    m["h0T"] = h0T
    m["h0T_16"] = _f16(h0T)
    r0 = np.zeros((1, 2, BL), f32)
    r0[:, 1, :] = 1.0
    m["rmu0"] = r0

    for d in "fb":
        wih = np.asarray(inp[f"enc_Wih_{d}"], f32)   # [768, 128]
        whh = np.asarray(inp[f"enc_Whh_{d}"], f32)   # [768, 256]
        bih = np.asarray(inp[f"enc_bih_{d}"], f32)
        bhh = np.asarray(inp[f"enc_bhh_{d}"], f32)
        m[f"enc_wihT_{d}"] = np.ascontiguousarray(wih.T)  # [128, 768]
        wt = whh.T.reshape(2, 128, 768).transpose(1, 0, 2)
        hi, lo = _hi_lo(wt)
        m[f"enc_whh_hi_{d}"] = np.ascontiguousarray(hi)
        m[f"enc_whh_lo_{d}"] = np.ascontiguousarray(lo)
        gb = np.concatenate([(bih + bhh)[:2 * D_ENC], bih[2 * D_ENC:]])
        m[f"enc_gbias_{d}"] = np.ascontiguousarray(gb[None, :])
        m[f"enc_bhn_{d}"] = _f16(bhh[2 * D_ENC:][None, :])

    dwih = np.asarray(inp["dec_Wih"], f32)  # [1536, 128]
    dwhh = np.asarray(inp["dec_Whh"], f32)  # [1536, 512]
    dbih = np.asarray(inp["dec_bih"], f32)
    dbhh = np.asarray(inp["dec_bhh"], f32)
    m["dec_wihT"] = np.ascontiguousarray(dwih.T)
    wt = dwhh.T.reshape(4, 128, 1536).transpose(1, 0, 2)
    hi, lo = _hi_lo(wt)
    m["dec_whh_hi"] = np.ascontiguousarray(hi)
    m["dec_whh_lo"] = np.ascontiguousarray(lo)
    m["dec_gbias"] = np.ascontiguousarray(np.concatenate(
        [(dbih + dbhh)[:2 * D_DEC], dbih[2 * D_DEC:]])[None, :])
    # u = row sums of Whh (for the LayerNorm-bypass correction)
    u = dwhh.sum(axis=1)                               # [1536]
    m["dec_uT"] = np.ascontiguousarray(u.reshape(12, 128).T)
    m["dec_bhnB"] = np.ascontiguousarray(np.broadcast_to(
        dbhh[2 * D_DEC:].reshape(4, 128).T[:, :, None], (128, 4, BL)))
    m["boutB"] = np.ascontiguousarray(np.broadcast_to(
        np.asarray(inp["attn_out_b"], f32).reshape(4, 128).T[:, :, None],
        (128, 4, BL)))

    aw = np.asarray(inp["attn_in_w"], f32)
    ab = np.asarray(inp["attn_in_b"], f32)
    Wq, Wk, Wv = aw[:512], aw[512:1024], aw[1024:]
    bq, bk, bv = ab[:512], ab[512:1024], ab[1024:]
    m["wqT"] = _f16(Wq.T.reshape(4, 128, 512).transpose(1, 0, 2))
    m["bq_row"] = _f16(bq[None, :])
    m["wkT"] = _f16(Wk.T.reshape(4, 128, 512).transpose(1, 0, 2))
    m["wvT"] = _f16(Wv.T.reshape(4, 128, 512).transpose(1, 0, 2))
    m["bk_row"] = _f16(bk[None, :])
    m["bv_row"] = _f16(bv[None, :])
    wout = np.asarray(inp["attn_out_w"], f32)
    wt = wout.T.reshape(4, 128, 512).transpose(1, 0, 2)
    hi, lo = _hi_lo(wt)
    m["wout_hi"] = np.ascontiguousarray(hi)
    m["wout_lo"] = np.ascontiguousarray(lo)
    mk = np.zeros((128, 2), f32)
    mk[:64, 0] = 1.0
    mk[64:, 1] = 1.0
    m["masks"] = _f16(mk)
    m["onesN16"] = _f16(np.ones((1, 512), f32))
    m["ones128_16"] = _f16(np.ones((128, 1), f32))
    m["ones512_r"] = np.ones((1, 512), f32)
    m["onesM_r"] = np.ones((1, 128), f32)
    m["ones128_r"] = np.ones((128, 1), f32)
    pw = np.asarray(inp["proj_w"], f32)  # [32000, 512]
    m["projT"] = _f16(pw.T.reshape(4, 128, V).transpose(1, 0, 2))
    m["projb"] = _f16(np.asarray(inp["proj_b"], f32)[None, :])
    return m


_PROGRAM_CACHE = {}


def kernel(**inputs):
    key = "full"
    if key not in _PROGRAM_CACHE:
        _PROGRAM_CACHE[key] = build_program()
    nc = _PROGRAM_CACHE[key]
    in_maps = [_prep_core_inputs(inputs, core) for core in range(NCORES)]
    res = run_bass_kernel_spmd(nc, in_maps, list(range(NCORES)))
    out = np.concatenate([res.results[i]["out_v"] for i in range(NCORES)], axis=0)
    return out.astype(np.float32)


if __name__ == "__main__":
    import pickle

    with open("/tmp/inputs.pkl", "rb") as f:
        inputs = pickle.load(f)
    out = kernel(**inputs)
    exp = np.load("/tmp/np_ref_out.npy")
    err = np.abs(out - exp)
    print("absmax", err.max(), "scale", np.abs(exp).max(),
          "rel", err.max() / np.abs(exp).max())


# revision 19
# speedup vs baseline: 1.1354x; 1.1354x over previous
"""Trainium2 Bass kernel for nn_DenoiseGRU (bidirectional-GRU encoder +
attention GRU decoder + vocab projection).

Strategy (8 NeuronCores, SPMD, data-parallel over batch, BL=4 rows/core):
  - fp16 stationaries everywhere on the Tensor engine (FWL single-pass;
    fp32 stationaries lower to 2x LOW/HIGH passes at ~1.4us each -- avoided).
  - Precision-critical weights (dec Whh, attn_out_w) as fp16 hi+lo dual-rail
    (two accumulating matmul arms ~= f32-grade).
  - LayerNorm bypass: Whh @ h_new == rstd*(Whh @ y) - rstd*mu*(Whh @ 1),
    so the GRU matmuls consume y directly and the LN stats/rsqrt chain runs
    off the critical path. ln_g == 1, ln_b == 0 (fixed by the reference).
  - Softmax normalization deferred past the ctx matmuls (runs in parallel).
  - Vocab projection (512 x 32000) in fp16 computed per 32-step block,
    interleaved 2 vocab-chunks per decoder step to fill PE gaps; fp16 DRAM
    output upconverted on host.

Self-contained: hardcodes all shapes; no sibling imports.
"""

import sys
from contextlib import ExitStack

import numpy as np

sys.path.insert(0, "/opt/trn_rl_repo")

import concourse.bass as bass
import concourse.bacc as bacc
import concourse.tile as tile
from concourse import mybir
from concourse.bass_utils import run_bass_kernel_spmd

F32 = mybir.dt.float32
F32R = mybir.dt.float32r
F16 = mybir.dt.float16
I32 = mybir.dt.int32
AF = mybir.ActivationFunctionType
ALU = mybir.AluOpType

D_E, D_ENC, D_DEC, H, HD = 128, 256, 512, 8, 64
B, S, T, V, C = 32, 128, 128, 32000, 2
NCORES = 8
BL = B // NCORES      # 4 local batch rows per core
VC = 500              # vocab chunk (PSUM bank = 500 f32)
NVC = V // VC         # 64
TBLK = 32             # decoder steps per projection block
NBLK = T // TBLK      # 4
HERON_ITERS = 2


def _f16(x):
    return np.asarray(x, np.float32).astype(np.float16)


def _hi_lo(x):
    """Split f32 array into fp16 hi + fp16 lo(residual)."""
    x = np.asarray(x, np.float32)
    hi = x.astype(np.float16)
    lo = (x - hi.astype(np.float32)).astype(np.float16)
    return hi, lo


def build_program(dec_steps=T, enc_steps=S, debug_outs=False, finalize=True):
    nc = bacc.Bacc()

    def din(name, shape, dtype=F32):
        return nc.dram_tensor(name, shape, dtype, kind="ExternalInput")

    # ---------------- DRAM inputs (per-core) ----------------
    embT = din("embT", [D_E, S, BL], F32R)     # encoder emb^T (d, s, b)
    embT_rev = din("embT_rev", [D_E, S, BL], F32R)  # s-reversed copy
    dembT = din("dembT", [D_E, T, BL], F32R)   # decoder input emb^T (d, t, b)
    h0T = din("h0T", [128, 4, BL])             # decoder h_init^T (p, c, b) f32
    h0T_16 = din("h0T_16", [128, 4, BL], F16)
    rmu0 = din("rmu0", [1, 2, BL])             # (mu=0, rstd=1) init

    enc_wihT = {d: din(f"enc_wihT_{d}", [D_E, 6 * 128], F32R) for d in "fb"}
    enc_whh_hi = {d: din(f"enc_whh_hi_{d}", [128, 2, 6 * 128], F16) for d in "fb"}
    enc_whh_lo = {d: din(f"enc_whh_lo_{d}", [128, 2, 6 * 128], F16) for d in "fb"}
    enc_gbias = {d: din(f"enc_gbias_{d}", [1, 6 * 128], F32R) for d in "fb"}
    enc_bhn = {d: din(f"enc_bhn_{d}", [1, 2 * 128], F16) for d in "fb"}

    dec_wihT = din("dec_wihT", [D_E, 12 * 128], F32R)
    dec_whh_hi = din("dec_whh_hi", [128, 4, 12 * 128], F16)
    dec_whh_lo = din("dec_whh_lo", [128, 4, 12 * 128], F16)
    dec_gbias = din("dec_gbias", [1, 12 * 128], F32R)
    dec_uT = din("dec_uT", [128, 12])          # Whh row sums (p, m) f32
    dec_bhnB = din("dec_bhnB", [128, 4, BL])   # bhh n-part broadcast f32
    boutB = din("boutB", [128, 4, BL])         # attn_out_b broadcast f32

    wqT = din("wqT", [128, 4, 512], F16)
    bq_row = din("bq_row", [1, 512], F16)
    wout_hi = din("wout_hi", [128, 4, 512], F16)
    wout_lo = din("wout_lo", [128, 4, 512], F16)
    wkT = din("wkT", [128, 4, 512], F16)
    wvT = din("wvT", [128, 4, 512], F16)
    bk_row = din("bk_row", [1, 512], F16)
    bv_row = din("bv_row", [1, 512], F16)
    masks = din("masks", [128, 2], F16)        # lo/hi head masks
    onesN16 = din("onesN16", [1, 512], F16)
    ones128_16 = din("ones128_16", [128, 1], F16)
    ones512_r = din("ones512_r", [1, 512], F32R)
    onesM_r = din("onesM_r", [1, 128], F32R)   # f32r ones row (bcast stat.)
    ones128_r = din("ones128_r", [128, 1], F32R)

    projT = din("projT", [128, 4, V], F16)     # proj_w^T packed (p, c, v)
    projb = din("projb", [1, V], F16)

    out_v = nc.dram_tensor("out_v", [BL, T, V], F16, kind="ExternalOutput")
    dbg = {}
    if debug_outs:
        dbg["mem"] = nc.dram_tensor("dbg_mem", [128, 4, BL, S], F16, kind="ExternalOutput")
        dbg["kT"] = nc.dram_tensor("dbg_kT", [128, 4, BL, S], F16, kind="ExternalOutput")
        dbg["v"] = nc.dram_tensor("dbg_v", [128, BL, 512], F16, kind="ExternalOutput")
        dbg["hnew"] = nc.dram_tensor("dbg_hnew", [128, 4, T, BL], F16, kind="ExternalOutput")

    with tile.TileContext(nc) as tc, ExitStack() as ctx:
        P = ctx.enter_context(tc.tile_pool(name="persist", bufs=1))
        W = ctx.enter_context(tc.tile_pool(name="work", bufs=3))
        PS = ctx.enter_context(tc.tile_pool(name="psum", bufs=1, space="PSUM"))
        PSB = ctx.enter_context(tc.tile_pool(name="psumb", bufs=2, space="PSUM"))
        PBG = ctx.enter_context(tc.tile_pool(name="psumg", bufs=2, space="PSUM"))
        PSJ = ctx.enter_context(tc.tile_pool(name="psumj", bufs=2, space="PSUM"))
        PWP = ctx.enter_context(tc.tile_pool(name="pw", bufs=4))

        dma = nc.sync.dma_start

        # ---------------- SBUF persistent tiles + loads ----------------
        def load(name, dram, shape, dtype):
            t = P.tile(shape, dtype, tag=name)
            dma(out=t, in_=dram)
            return t

        s_embT = load("s_embT", embT[:, :, :], [D_E, S, BL], F32R)
        s_embT_rev = load("s_embT_rev", embT_rev[:, :, :], [D_E, S, BL], F32R)
        s_dembT = load("s_dembT", dembT[:, :, :], [D_E, T, BL], F32R)
        s_h0 = load("s_h0", h0T[:, :, :], [128, 4, BL], F32)
        s_h016 = load("s_h016", h0T_16[:, :, :], [128, 4, BL], F16)
        s_rmu0 = load("s_rmu0", rmu0[:, :, :], [1, 2, BL], F32)
        s_ewihT = {d: load(f"s_ewihT_{d}", enc_wihT[d][:, :], [D_E, 768], F32R) for d in "fb"}
        s_ewhh_hi = {d: load(f"s_ewhh_hi_{d}", enc_whh_hi[d][:, :, :], [128, 2, 768], F16) for d in "fb"}
        s_ewhh_lo = {d: load(f"s_ewhh_lo_{d}", enc_whh_lo[d][:, :, :], [128, 2, 768], F16) for d in "fb"}
        s_egb = {d: load(f"s_egb_{d}", enc_gbias[d][:, :], [1, 768], F32R) for d in "fb"}
        s_ebhn = {d: load(f"s_ebhn_{d}", enc_bhn[d][:, :], [1, 256], F16) for d in "fb"}
        s_dwihT = load("s_dwihT", dec_wihT[:, :], [D_E, 1536], F32R)
        s_dwhh_hi = load("s_dwhh_hi", dec_whh_hi[:, :, :], [128, 4, 1536], F16)
        s_dwhh_lo = load("s_dwhh_lo", dec_whh_lo[:, :, :], [128, 4, 1536], F16)
        s_dgb = load("s_dgb", dec_gbias[:, :], [1, 1536], F32R)
        s_u = load("s_u", dec_uT[:, :], [128, 12], F32)
        s_bhnB = load("s_bhnB", dec_bhnB[:, :, :], [128, 4, BL], F32)
        s_boutB = load("s_boutB", boutB[:, :, :], [128, 4, BL], F32)
        s_wq = load("s_wq", wqT[:, :, :], [128, 4, 512], F16)
        s_bq = load("s_bq", bq_row[:, :], [1, 512], F16)
        s_wout_hi = load("s_wout_hi", wout_hi[:, :, :], [128, 4, 512], F16)
        s_wout_lo = load("s_wout_lo", wout_lo[:, :, :], [128, 4, 512], F16)
        s_wk = load("s_wk", wkT[:, :, :], [128, 4, 512], F16)
        s_wv = load("s_wv", wvT[:, :, :], [128, 4, 512], F16)
        s_bk = load("s_bk", bk_row[:, :], [1, 512], F16)
        s_bv = load("s_bv", bv_row[:, :], [1, 512], F16)
        s_masks = load("s_masks", masks[:, :], [128, 2], F16)
        s_onesN = load("s_onesN", onesN16[:, :], [1, 512], F16)
        s_ones128_16 = load("s_ones128_16", ones128_16[:, :], [128, 1], F16)
        s_ones512_r = load("s_ones512_r", ones512_r[:, :], [1, 512], F32R)
        s_onesM_r = load("s_onesM_r", onesM_r[:, :], [1, 128], F32R)
        s_ones128_r = load("s_ones128_r", ones128_r[:, :], [128, 1], F32R)

        # ---- persistent state/stores ----
        s_giE = P.tile([128, 12, S, BL], F16, tag="s_giE")
        POS = {"f": [0, 1, 4, 5, 8, 9], "b": [2, 3, 6, 7, 10, 11]}
        s_gid = P.tile([128, 12, T, BL], F32, tag="s_gid")
        s_mem16 = P.tile([128, 4, BL, S], F16, tag="s_mem16")   # (p, c, b, s)
        s_kT = P.tile([128, 4, BL, S], F16, tag="s_kT")
        s_v16 = P.tile([128, BL, 512], F16, tag="s_v16")        # (p=s, b, vdim)
        s_hc = [P.tile([128, 4, BL], F16, name=f"s_hc{i}", tag=f"s_hc{i}")
                for i in range(2)]                              # enc carry ping-pong
        s_h = P.tile([128, 4, BL], F32, tag="s_h")              # h_new[t-1] f32
        s_y16d = P.tile([128, 4, 2 * BL], F16, tag="s_y16d")    # y | y_lo fp16
        s_ys2 = P.tile([128, 2, 4, BL], F32, tag="s_ys2")       # y, y^2
        s_ysr = P.tile([128, 2, 4, BL], F32, tag="s_ysr")       # y, y^2 (f32r)
        s_rmu = P.tile([1, 2, BL], F32, tag="s_rmu")            # (mu, rstd)
        s_rmu16h = P.tile([1, 2, BL], F16, tag="s_rmu16h")
        s_rmu16l = P.tile([1, 2, BL], F16, tag="s_rmu16l")
        s_hb = [P.tile([128, 4, TBLK, BL], F16, name=f"s_hb{i}", tag=f"s_hb{i}")
                for i in range(NBLK)]
        s_fisK = P.tile([1, BL], I32, tag="s_fisK")

        MM = nc.tensor.matmul
        TT = nc.vector.tensor_tensor
        CP = nc.vector.tensor_copy
        STT = nc.vector.scalar_tensor_tensor

        def TS(out, in0, s1, s2, op0, op1=None):
            if op1 is None:
                nc.vector.tensor_scalar(out, in0, s1, None, op0)
            else:
                nc.vector.tensor_scalar(out, in0, s1, s2, op0, op1)

        def r32(ap):
            return ap.bitcast(F32R)

        def mkap(ap_, free_dims):
            """Replace the free dims of a (tile-slice) AP with explicit
            [step, count] entries (steps in elements; 0 = broadcast)."""
            return bass.AP(tensor=ap_.tensor, offset=ap_.offset,
                           ap=[list(ap_.ap[0])] + [list(x) for x in free_dims])

        nc.vector._memset_packed(s_fisK, 0x5F3759DF)
        nc.vector._memset_packed(
            s_hc[1].rearrange("p c b -> p (c b)").bitcast(I32), 0)

        # ================ phase 1: gi builds (f32r matmuls -> fp16 stores) ==
        def build_gi(wihT_s, gb_s, src, dest, nm, nt, posmap=None):
            # dest[:, m, t, b] = (Wih @ x^T)[128m:128m+128, (t,b)] + bias
            for m in range(nm):
                pb = PBG.tile([128, nt * BL], F32, name="pb", tag="pbig")
                MM(pb, gb_s[:, 128 * m:128 * (m + 1)],
                   s_ones512_r[:, :nt * BL], start=True, stop=False)
                MM(pb, wihT_s[:, 128 * m:128 * (m + 1)],
                   src.rearrange("d t b -> d (t b)"), start=False, stop=True)
                mo = posmap[m] if posmap else m
                CP(dest[:, mo, :, :].rearrange("p t b -> p (t b)"), pb)

        build_gi(s_ewihT["f"], s_egb["f"], s_embT, s_giE, 6, S, POS["f"])
        build_gi(s_ewihT["b"], s_egb["b"], s_embT_rev, s_giE, 6, S, POS["b"])
        build_gi(s_dwihT, s_dgb, s_dembT, s_gid, 12, T)

        # ================ phase 2: encoder scans ================
        def enc_mms(d, t, pghe, hprev):
            # matmuls for one direction into the merged psum layout:
            # positions [f_r b_r f_z b_z f_n b_n] (2 chunks each)
            whh_hi = s_ewhh_hi[d]
            off = 0 if d == "f" else 2
            hmov = hprev[:, off:off + 2, :]
            pos = POS[d]
            for m in range(6):
                mo = pos[m]
                if m >= 4:
                    MM(pghe[:, mo, :], s_ebhn[d][:, 128 * (m - 4):128 * (m - 3)],
                       s_onesN[:, :BL], start=True, stop=False)
                for k in range(2):
                    MM(pghe[:, mo, :], whh_hi[:, k, 128 * m:128 * (m + 1)],
                       hmov[:, k, :],
                       start=(m < 4 and k == 0), stop=(k == 1))

        def enc_gates(t, pghe, hprev, hcur):
            pre8 = W.tile([128, 8, BL], F32, tag="e_pre8")
            TT(pre8, pghe[:, 0:8, :], s_giE[:, 0:8, t, :], ALU.add)
            rzt = W.tile([128, 8, BL], F32, tag="e_rzt")
            nc.scalar.activation(rzt, pre8, AF.Tanh, scale=0.5)
            tmp4 = W.tile([128, 4, BL], F32, tag="e_tmp4")
            STT(tmp4, rzt[:, 0:4, :], 1.0, pghe[:, 8:12, :], ALU.add, ALU.mult)
            tn4 = W.tile([128, 4, BL], F32, tag="e_tn4")
            STT(tn4, tmp4, 0.5, s_giE[:, 8:12, t, :], ALU.mult, ALU.add)
            nn4 = W.tile([128, 4, BL], F32, tag="e_nn4")
            nc.scalar.activation(nn4, tn4, AF.Tanh)
            dd = W.tile([128, 4, BL], F32, tag="e_dd")
            TT(dd, hprev, nn4, ALU.subtract)
            STT(dd, rzt[:, 4:8, :], 1.0, dd, ALU.add, ALU.mult)
            STT(hcur, dd, 0.5, nn4, ALU.mult, ALU.add)
            # off-chain: fp16 memory stores for k/v (not on the recurrence)
            CP(s_mem16[:, 0:2, :, t], hcur[:, 0:2, :])
            CP(s_mem16[:, 2:4, :, S - 1 - t], hcur[:, 2:4, :])

        if enc_steps < S:
            nc.vector._memset_packed(
                s_mem16.rearrange("p c b s -> p (c b s)").bitcast(I32), 0)
        for t in range(enc_steps):
            pghe = PS.tile([128, 12, BL], F32, name="pghe", tag="pghe")
            hprev = s_hc[(t + 1) % 2]
            hcur = s_hc[t % 2]
            enc_mms("f", t, pghe, hprev)
            enc_mms("b", t, pghe, hprev)
            enc_gates(t, pghe, hprev, hcur)

        # ================ phase 3: k / v builds ================
        for m in range(4):
            pb = PBG.tile([128, BL * S], F32, name="pb", tag="pbig")
            MM(pb, s_bk[:, 128 * m:128 * (m + 1)], s_onesN, start=True, stop=False)
            for k in range(4):
                MM(pb, s_wk[:, k, 128 * m:128 * (m + 1)],
                   s_mem16[:, k, :, :].rearrange("p b s -> p (b s)"),
                   start=False, stop=(k == 3))
            CP(s_kT[:, m, :, :].rearrange("p b s -> p (b s)"), pb)
            if debug_outs:
                dma(out=dbg["kT"][:, m, :, :], in_=s_kT[:, m, :, :])
        for g in range(BL):
            pb = PBG.tile([128, 512], F32, name="pb", tag="pbig")
            MM(pb, s_onesN[:, 0:128], s_bv, start=True, stop=False)
            for k in range(4):
                MM(pb, s_mem16[:, k, g, :], s_wv[:, k, :], start=False, stop=(k == 3))
            CP(s_v16[:, g, :], pb)
            if debug_outs:
                dma(out=dbg["v"][:, g, :], in_=s_v16[:, g, :])
        if debug_outs:
            dma(out=dbg["mem"].rearrange("p c b s -> p (c b s)"),
                in_=s_mem16.rearrange("p c b s -> p (c b s)"))

        # ================ phase 4+5: decoder + interleaved projection =======
        RECIP = nc.vector.reciprocal

        # init: y[-1] := h0 (with mu=0, rstd=1 so that h_new[-1] = h0)
        CP(s_h, s_h0)
        CP(s_y16d[:, :, 0:BL], s_h016)
        TT(s_y16d[:, :, BL:2 * BL], s_h0, s_h016, ALU.subtract)
        CP(s_rmu, s_rmu0)

        def gru_mms(t, m0, m1):
            # arm0: Whh_hi @ [y | y_lo] (N=8) draining both col groups onto the
            # same PSUM elements (step-0 output AP accumulates); arm1: lo @ y
            pgh = dec_state["pgh"]
            for m in range(m0, m1):
                out0 = mkap(pgh[:, m, 0:BL], [[0, 2], [1, BL]])
                for k in range(4):
                    MM(out0, s_dwhh_hi[:, k, 128 * m:128 * (m + 1)],
                       s_y16d[:, k, :].rearrange("p (j b) -> p j b", j=2),
                       start=(k == 0), stop=False)
                for k in range(4):
                    MM(pgh[:, m, 0:BL], s_dwhh_lo[:, k, 128 * m:128 * (m + 1)],
                       s_y16d[:, k, 0:BL], start=False, stop=(k == 3))

        def stats_mms(pbcx):
            # partition-sums of (y, y^2) for the previous step
            pst2 = pbcx[0:1, 40:48].rearrange("p (a b) -> p a b", a=2)
            for c in range(4):
                MM(pst2, s_ones128_r, r32(s_ysr[:, :, c, :]),
                   start=(c == 0), stop=(c == 3))
            return pst2

        def stats_chain(pst2):
            # mu, rstd <- pst2  (DVE, off critical path)
            TS(s_rmu[:, 0, :], pst2[:, 0, :], 1.0 / D_DEC, None, ALU.mult)
            mu2 = W.tile([1, BL], F32, tag="d_mu2")
            STT(mu2, s_rmu[:, 0, :], 1.0, s_rmu[:, 0, :], ALU.mult, ALU.mult)
            varE = W.tile([1, BL], F32, tag="d_varE")
            STT(varE, pst2[:, 1, :], 1.0 / D_DEC, mu2, ALU.mult, ALU.subtract)
            zz = W.tile([1, BL], F32, tag="d_zz")
            TS(zz.bitcast(I32), varE.bitcast(I32), 1, None,
               ALU.logical_shift_right)
            TT(zz.bitcast(I32), s_fisK.bitcast(I32), zz.bitcast(I32),
               ALU.subtract)
            dtmp = W.tile([1, BL], F32, tag="d_dtmp")
            for it in range(HERON_ITERS):
                TT(dtmp, zz, zz, ALU.mult)
                TT(dtmp, varE, dtmp, ALU.mult)
                TS(dtmp, dtmp, -0.5, 1.5, ALU.mult, ALU.add)
                if it == HERON_ITERS - 1:
                    TT(s_rmu[:, 1, :], zz, dtmp, ALU.mult)
                else:
                    TT(zz, zz, dtmp, ALU.mult)

        def bcast_rmu(pbcx):
            CP(s_rmu16h, s_rmu)
            TT(s_rmu16l, s_rmu, s_rmu16h, ALU.subtract)
            pbc2 = pbcx[:, 0:8].rearrange("p (a b) -> p a b", a=2)
            MM(pbc2.rearrange("p a b -> p (a b)"), s_onesN[:, 0:128],
               s_rmu16h.rearrange("p a b -> p (a b)"), start=True, stop=False)
            MM(pbc2.rearrange("p a b -> p (a b)"), s_onesN[:, 0:128],
               s_rmu16l.rearrange("p a b -> p (a b)"), start=False, stop=True)
            return pbc2

        def materialize_h(t, rmuB):
            # h_new[t] = (y[t] - mu)*rstd  -> s_h (f32) and block store (fp16)
            ytmp = W.tile([128, 4, BL], F32, tag="d_ytmp")
            TT(ytmp, s_ys2[:, 0, :, :], mkap(rmuB[:, 0, :], [[0, 4], [1, BL]]),
               ALU.subtract)
            TT(s_h, ytmp, mkap(rmuB[:, 1, :], [[0, 4], [1, BL]]), ALU.mult)
            CP(s_hb[t // TBLK][:, :, t % TBLK, :], s_h)

        def proj_vc(blk, vc):
            s_pw = PWP.tile([128, 4, VC], F16, tag="s_pw")
            dma(out=s_pw, in_=projT[:, :, VC * vc:VC * (vc + 1)])
            s_pb = PWP.tile([1, VC], F16, tag="s_pb")
            dma(out=s_pb, in_=projb[:, VC * vc:VC * (vc + 1)])
            pj = PSJ.tile([128, VC], F32, tag="pproj")
            MM(pj, s_onesN[:, 0:128], s_pb, start=True, stop=False)
            for c in range(4):
                MM(pj, s_hb[blk][:, c, :, :].rearrange("p t b -> p (t b)"),
                   s_pw[:, c, :], start=False, stop=(c == 3))
            so = PWP.tile([128, VC], F16, tag="s_out")
            CP(so, pj)
            r0 = blk * TBLK
            dma(out=out_v[:, r0:r0 + TBLK, VC * vc:VC * (vc + 1)]
                .rearrange("b t v -> t b v"), in_=so)

        dec_state = {}
        for t in range(dec_steps):
            # ---- GRU matmuls on y[t-1] (start immediately; only need y16) --
            pgh = PS.tile([128, 12, BL], F32, name="pgh", tag="pghe")
            dec_state["pgh"] = pgh
            patt = PS.tile([128, 160], F32, name="patt", tag="patt")
            pbcx = PSB.tile([128, 64], F32, name="pbcx", tag="pbcx")
            gru_mms(t, 0, 4)
            pst2 = stats_mms(pbcx) if t > 0 else None
            gru_mms(t, 4, 10)
            if t > 0:
                stats_chain(pst2)           # DVE, overlaps the m=4..9 MMs
            pbc2 = bcast_rmu(pbcx)          # PE; waits on s_rmu (newton)
            gru_mms(t, 10, 12)
            rmuB = W.tile([128, 2, BL], F32, name="rmuB", tag="d_rmuB")
            CP(rmuB, pbc2)
            muB = rmuB[:, 0, :]
            rstdB = rmuB[:, 1, :]
            # ---- corrections (DVE, overlap the GRU tail) ----
            rm = W.tile([128, BL], F32, tag="d_rm")
            TT(rm, muB, rstdB, ALU.mult)
            uw = W.tile([128, 12, BL], F32, tag="d_uw")
            TT(uw, mkap(s_u, [[1, 12], [0, BL]]),
               mkap(rm, [[0, 12], [1, BL]]), ALU.mult)
            giA = W.tile([128, 8, BL], F32, tag="d_giA")
            TT(giA, s_gid[:, 0:8, t, :], uw[:, 0:8, :], ALU.subtract)
            Cn = W.tile([128, 4, BL], F32, tag="d_Cn")
            TT(Cn, s_bhnB, uw[:, 8:12, :], ALU.subtract)
            if t > 0:
                materialize_h(t - 1, rmuB)
            # ---- gates ----
            pghS = W.tile([128, 12, BL], F32, tag="d_pghS")
            TT(pghS, pgh[:, :, 0:BL], mkap(rstdB, [[0, 12], [1, BL]]), ALU.mult)
            pre8 = W.tile([128, 8, BL], F32, tag="d_pre8")
            TT(pre8, pghS[:, 0:8, :], giA, ALU.add)
            npre = W.tile([128, 4, BL], F32, tag="d_npre")
            TT(npre, pghS[:, 8:12, :], Cn, ALU.add)
            rzt = W.tile([128, 8, BL], F32, tag="d_rzt")
            nc.scalar.activation(rzt, pre8, AF.Tanh, scale=0.5)
            tn = W.tile([128, 4, BL], F32, tag="d_tn")
            STT(tn, rzt[:, 0:4, :], 1.0, npre, ALU.add, ALU.mult)
            STT(tn, tn, 0.5, s_gid[:, 8:12, t, :], ALU.mult, ALU.add)
            nn_ = W.tile([128, 4, BL], F32, tag="d_n")
            nc.scalar.activation(nn_, tn, AF.Tanh)
            dd = W.tile([128, 4, BL], F32, tag="d_dd")
            TT(dd, s_h, nn_, ALU.subtract)
            STT(dd, rzt[:, 4:8, :], 1.0, dd, ALU.add, ALU.mult)
            hdecT = W.tile([128, 4, BL], F32, tag="d_hdec")
            STT(hdecT, dd, 0.5, nn_, ALU.mult, ALU.add)
            hdec16 = W.tile([128, 4, BL], F16, tag="d_hdec16")
            CP(hdec16, hdecT)
            # ---- q = Wq @ h_dec + bq ----
            pq = patt[:, 0:16].rearrange("p (c b) -> p c b", c=4)
            for m in range(4):
                MM(pq[:, m, :], s_bq[:, 128 * m:128 * (m + 1)],
                   s_onesN[:, :BL], start=True, stop=False)
                for k in range(4):
                    MM(pq[:, m, :], s_wq[:, k, 128 * m:128 * (m + 1)],
                       hdec16[:, k, :], start=False, stop=(k == 3))
            # ---- masked q ----
            mq = W.tile([128, 4, BL, 2], F16, tag="d_mq")
            TT(mq, pq.to_broadcast([128, 4, BL, 2]),
               mkap(s_masks, [[0, 4], [0, BL], [1, 2]]), ALU.mult)
            # ---- scores^T ----
            psc = patt[:, 16:48].rearrange("p (b c j) -> p b c j", b=BL, c=4)
            for b in range(BL):
                for c in range(4):
                    MM(psc[:, b, c, :], s_kT[:, c, b, :], mq[:, c, b, :],
                       start=True, stop=True)
            # ---- exp (unnormalized); denominator+recip run beside ctx ----
            attnE = W.tile([128, BL, 4, 2], F16, tag="d_attnE")
            nc.scalar.activation(attnE.rearrange("p b c j -> p (b c j)"),
                                 psc.rearrange("p b c j -> p (b c j)"),
                                 AF.Exp, scale=0.125)
            if t >= TBLK:
                proj_vc(t // TBLK - 1, 2 * (t % TBLK))
            pst = patt[0:1, 48:80]
            MM(pst, s_ones128_16, attnE.rearrange("p b c j -> p (b c j)"),
               start=True, stop=True)
            rec = W.tile([1, 32], F32, tag="d_rec")
            with nc.allow_low_precision(reason="f32r recip for bcast mm"):
                RECIP(r32(rec), pst)
            pbc = pbcx[:, 8:40]
            MM(pbc, s_onesM_r, r32(rec), start=True, stop=True)
            recB = W.tile([128, BL, 4, 2], F32, tag="d_recB")
            CP(recB.rearrange("p b c j -> p (b c j)"), pbc)
            pctx = patt[:, 80:112].rearrange("p (b c j) -> p b c j", b=BL, c=4)
            for b in range(BL):
                for c in range(4):
                    MM(pctx[:, b, c, :], s_v16[:, b, 128 * c:128 * (c + 1)],
                       attnE[:, b, c, :], start=True, stop=True)
            # ---- normalize + head-merge: ctxb[p, c, b] (fp16) ----
            ctxf = W.tile([128, 4, BL], F32, tag="d_ctxf")
            TT(ctxf[0:64, :, :].rearrange("p c b -> p b c"),
               pctx[0:64, :, :, 0], recB[0:64, :, :, 0], ALU.mult)
            TT(ctxf[64:128, :, :].rearrange("p c b -> p b c"),
               pctx[64:128, :, :, 1], recB[64:128, :, :, 1], ALU.mult)
            ctxd = W.tile([128, 4, 2 * BL], F16, tag="d_ctxd")
            CP(ctxd[:, :, 0:BL], ctxf)
            TT(ctxd[:, :, BL:2 * BL], ctxf, ctxd[:, :, 0:BL], ALU.subtract)
            # ---- c = Wout @ ctx: hi @ [ctx|ctx_lo] (N=8) + lo @ ctx (N=4) ----
            pc = patt[:, 112:128].rearrange("p (c b) -> p c b", c=4)
            for m in range(4):
                outc = mkap(pc[:, m, :], [[0, 2], [1, BL]])
                for k in range(4):
                    MM(outc, s_wout_hi[:, k, 128 * m:128 * (m + 1)],
                       ctxd[:, k, :].rearrange("p (j b) -> p j b", j=2),
                       start=(k == 0), stop=False)
                for k in range(4):
                    MM(pc[:, m, 0:BL], s_wout_lo[:, k, 128 * m:128 * (m + 1)],
                       ctxd[:, k, 0:BL], start=False, stop=(k == 3))
            # ---- y = hdec + c + bout ; y^2 ----
            TT(s_ys2[:, 0, :, :], hdecT, pc, ALU.add)
            TT(s_ys2[:, 0, :, :], s_ys2[:, 0, :, :], s_boutB, ALU.add)
            CP(s_y16d[:, :, 0:BL], s_ys2[:, 0, :, :])
            TT(s_y16d[:, :, BL:2 * BL], s_ys2[:, 0, :, :],
               s_y16d[:, :, 0:BL], ALU.subtract)
            TT(r32(s_ysr[:, 1, :, :]), s_ys2[:, 0, :, :], s_ys2[:, 0, :, :], ALU.mult)
            CP(r32(s_ysr[:, 0, :, :]), s_ys2[:, 0, :, :])
            # ---- interleaved projection: second chunk at step end ----
            if t >= TBLK:
                proj_vc(t // TBLK - 1, 2 * (t % TBLK) + 1)

        # ---- epilogue: stats for final y, materialize last h, last block ---
        pbcx = PSB.tile([128, 64], F32, name="pbcx", tag="pbcx")
        pst2 = stats_mms(pbcx)
        stats_chain(pst2)
        pbc2 = bcast_rmu(pbcx)
        rmuB = W.tile([128, 2, BL], F32, name="rmuB_f", tag="d_rmuB")
        CP(rmuB, pbc2)
        materialize_h(dec_steps - 1, rmuB)
        lastblk = (dec_steps - 1) // TBLK
        for vc in range(NVC):
            proj_vc(lastblk, vc)
        if debug_outs:
            for i in range(NBLK):
                dma(out=dbg["hnew"][:, :, TBLK * i:TBLK * (i + 1), :],
                    in_=s_hb[i])

    if finalize:
        nc.compile()
    return nc


# ---------------------------------------------------------------------------
# Host driver
# ---------------------------------------------------------------------------

def _prep_core_inputs(inp, core):
    """Build the per-core input map (numpy) for batch slice [4*core, 4*core+4)."""
    f32 = np.float32
    sl = slice(BL * core, BL * (core + 1))
    nx = np.asarray(inp["nx"])[sl]
    x = np.asarray(inp["x"])[sl]
    label = np.asarray(inp["label"])[sl]
    tok = np.asarray(inp["tok_emb"], f32)

    m = {}
    emb = tok[nx]  # [BL, S, D_E]
    m["embT"] = np.ascontiguousarray(emb.transpose(2, 1, 0))  # (d, s, b)
    m["embT_rev"] = np.ascontiguousarray(emb[:, ::-1].transpose(2, 1, 0))
    dec_in = np.concatenate(
        [np.broadcast_to(np.asarray(inp["start_emb"], f32)[None], (BL, 1, D_E)),
         tok[x[:, :-1]]], axis=1)
    m["dembT"] = np.ascontiguousarray(dec_in.transpose(2, 1, 0))
    h0 = np.asarray(inp["style_emb"], f32)[label]  # [BL, 512]
    h0T = np.ascontiguousarray(h0.T.reshape(4, 128, BL).transpose(1, 0, 2))
    m["h0T"] = h0T
    m["h0T_16"] = _f16(h0T)
    r0 = np.zeros((1, 2, BL), f32)
    r0[:, 1, :] = 1.0
    m["rmu0"] = r0

    for d in "fb":
        wih = np.asarray(inp[f"enc_Wih_{d}"], f32)   # [768, 128]
        whh = np.asarray(inp[f"enc_Whh_{d}"], f32)   # [768, 256]
        bih = np.asarray(inp[f"enc_bih_{d}"], f32)
        bhh = np.asarray(inp[f"enc_bhh_{d}"], f32)
        m[f"enc_wihT_{d}"] = np.ascontiguousarray(wih.T)  # [128, 768]
        wt = whh.T.reshape(2, 128, 768).transpose(1, 0, 2)
        hi, lo = _hi_lo(wt)
        m[f"enc_whh_hi_{d}"] = np.ascontiguousarray(hi)
        m[f"enc_whh_lo_{d}"] = np.ascontiguousarray(lo)
        gb = np.concatenate([(bih + bhh)[:2 * D_ENC], bih[2 * D_ENC:]])
        m[f"enc_gbias_{d}"] = np.ascontiguousarray(gb[None, :])
        m[f"enc_bhn_{d}"] = _f16(bhh[2 * D_ENC:][None, :])

    dwih = np.asarray(inp["dec_Wih"], f32)  # [1536, 128]
    dwhh = np.asarray(inp["dec_Whh"], f32)  # [1536, 512]
    dbih = np.asarray(inp["dec_bih"], f32)
    dbhh = np.asarray(inp["dec_bhh"], f32)
    m["dec_wihT"] = np.ascontiguousarray(dwih.T)
    wt = dwhh.T.reshape(4, 128, 1536).transpose(1, 0, 2)
    hi, lo = _hi_lo(wt)
    m["dec_whh_hi"] = np.ascontiguousarray(hi)
    m["dec_whh_lo"] = np.ascontiguousarray(lo)
    m["dec_gbias"] = np.ascontiguousarray(np.concatenate(
        [(dbih + dbhh)[:2 * D_DEC], dbih[2 * D_DEC:]])[None, :])
    # u = row sums of Whh (for the LayerNorm-bypass correction)
    u = dwhh.sum(axis=1)                               # [1536]
    m["dec_uT"] = np.ascontiguousarray(u.reshape(12, 128).T)
    m["dec_bhnB"] = np.ascontiguousarray(np.broadcast_to(
        dbhh[2 * D_DEC:].reshape(4, 128).T[:, :, None], (128, 4, BL)))
    m["boutB"] = np.ascontiguousarray(np.broadcast_to(
        np.asarray(inp["attn_out_b"], f32).reshape(4, 128).T[:, :, None],
        (128, 4, BL)))

    aw = np.asarray(inp["attn_in_w"], f32)
    ab = np.asarray(inp["attn_in_b"], f32)
    Wq, Wk, Wv = aw[:512], aw[512:1024], aw[1024:]
    bq, bk, bv = ab[:512], ab[512:1024], ab[1024:]
    m["wqT"] = _f16(Wq.T.reshape(4, 128, 512).transpose(1, 0, 2))
    m["bq_row"] = _f16(bq[None, :])
    m["wkT"] = _f16(Wk.T.reshape(4, 128, 512).transpose(1, 0, 2))
    m["wvT"] = _f16(Wv.T.reshape(4, 128, 512).transpose(1, 0, 2))
    m["bk_row"] = _f16(bk[None, :])
    m["bv_row"] = _f16(bv[None, :])
    wout = np.asarray(inp["attn_out_w"], f32)
    wt = wout.T.reshape(4, 128, 512).transpose(1, 0, 2)
    hi, lo = _hi_lo(wt)
    m["wout_hi"] = np.ascontiguousarray(hi)
    m["wout_lo"] = np.ascontiguousarray(lo)
    mk = np.zeros((128, 2), f32)
    mk[:64, 0] = 1.0
    mk[64:, 1] = 1.0
    m["masks"] = _f16(mk)
    m["onesN16"] = _f16(np.ones((1, 512), f32))
    m["ones128_16"] = _f16(np.ones((128, 1), f32))
    m["ones512_r"] = np.ones((1, 512), f32)
    m["onesM_r"] = np.ones((1, 128), f32)
    m["ones128_r"] = np.ones((128, 1), f32)
    pw = np.asarray(inp["proj_w"], f32)  # [32000, 512]
    m["projT"] = _f16(pw.T.reshape(4, 128, V).transpose(1, 0, 2))
    m["projb"] = _f16(np.asarray(inp["proj_b"], f32)[None, :])
    return m


_PROGRAM_CACHE = {}


def kernel(**inputs):
    key = "full"
    if key not in _PROGRAM_CACHE:
        _PROGRAM_CACHE[key] = build_program()
    nc = _PROGRAM_CACHE[key]
    in_maps = [_prep_core_inputs(inputs, core) for core in range(NCORES)]
    res = run_bass_kernel_spmd(nc, in_maps, list(range(NCORES)))
    out = np.concatenate([res.results[i]["out_v"] for i in range(NCORES)], axis=0)
    return out.astype(np.float32)


if __name__ == "__main__":
    import pickle

    with open("/tmp/inputs.pkl", "rb") as f:
        inputs = pickle.load(f)
    out = kernel(**inputs)
    exp = np.load("/tmp/np_ref_out.npy")
    err = np.abs(out - exp)
    print("absmax", err.max(), "scale", np.abs(exp).max(),
          "rel", err.max() / np.abs(exp).max())
